# revision 27
# baseline (speedup 1.0000x reference)
"""Canny edge detector (32,1,1024,1024) on 8 Trainium2 NeuronCores.

Strategy (per core: 4 images, data-parallel over batch; ~566us/core HW):
  - Host-side quantize: xq = floor(255*img) as fp16 (exact integers 0..255),
    fed as the kernel input (halves input DMA). Output is fp16 {0,1}, cast
    to fp32 on host (halves output DMA).
  - Row-tiles of 128 partitions: partition p holds image row r0+p-4
    (4-row top halo in partitions 0..3, so a mid tile is ONE contiguous
    DMA of 128 rows). Output rows r0..r0+119 live in partitions 4..123.
  - Blur = 10 accumulated band matmuls per 512-col half (fp16 hi/lo weight
    split, fp32 PSUM -> near-exact). Blur splits to an fp16 hi/lo pair via
    a PE trick: bh = ACT copy(psum); psum -= I@bh; bl = ACT copy(psum).
  - Sobel gy first (6 matmuls/half) -> gys = ACT evac -> gx reuses the
    same PSUM banks (4 matmuls/half). Custom DVE ops read (gys SBUF,
    ps_gx PSUM): mag, posm, c0m, c2m.
  - NMS in fp32 (fp16 NMS measured 11k+ flips vs ~4.4k budget): mag row
    shifts via fp32 identity-band matmuls in 512-col chunks with their own
    1-bank PSUM pool, ACT evacuation; pair maxes + 3 copy_predicated bin
    selects + fused keep custom op (fp16 {0,1} out).
  - Engine placement: ACT: all PSUM evacuations + out-DMA trigger. Pool:
    reflect pad columns + edge memsets only (HW pool is slow at bulk
    elementwise work). DVE: the 2-tensor fp32 NMS chain only; xq2 comes
    precomputed from the host so the PE never waits on the DVE queue.
  - DVE emission order matters (engines are in-order): mag is emitted
    FIRST so the PE shift matmuls unblock after one DVE op; the mask ops
    and mew then fill the DVE while the PE runs the fp32 shifts.
  - PSUM: blur pool bufs=2 (4 banks) + gy/gx shared pool (2) + shift
    chunks bufs=2 (2) = 8 banks exactly.
"""

import numpy as np

import concourse.bacc as bacc
import concourse.tile as tile
import concourse.mybir as mybir
from concourse import bass_utils
from concourse import dve_ops as _DO
from concourse.dve_spec import Spec, Src0, Src1, C0, Zero, maxx, lower as _dve_lower
from concourse.dve_uop import DveOpSpec as _DveOpSpec


def _register_custom_op(name, body, ref):
    """Runtime-register a fused DVE op (sha self-computed, v3/TRN2)."""
    if name in _DO._SUB_OPCODE_FOR_NAME:
        return next(op for op in _DO.OPS if op.name == name)
    op = _DO.DveOp(name, Spec(body=body, reference=ref), subdim=False, uops_sha={})
    _DO.OPS.append(op)
    _DO.CUSTOM_DVE_SPECS[name] = op.spec
    _DO._SUB_OPCODE_FOR_NAME[name] = _DO._CUSTOM_DVE_ROW_BASE + len(_DO.OPS) - 1
    for ver in ("v3",):
        compiled = _DveOpSpec(
            name=name,
            opcode=_DO.get_dve_sub_opcode(name),
            uops=_dve_lower(op.spec, ver=ver),
            rd1_en=True,
        )
        op.uops_sha[ver] = compiled.sha(ver)
    return op


_ABS0 = maxx(Src0, Zero - Src0)
_ABS1 = maxx(Src1, Zero - Src1)
OP_MAG = _register_custom_op(
    "CANNY_MAG", _ABS0 + _ABS1,
    lambda in0, in1, s0, s1, imm2: np.abs(in0) + np.abs(in1))
OP_POS = _register_custom_op(
    "CANNY_POS", (Src0 * Src1) > Zero,
    lambda in0, in1, s0, s1, imm2: (in0 * in1 > 0).astype(np.float32))
OP_C0 = _register_custom_op(
    "CANNY_C0", (_ABS0 * C0) < _ABS1,
    lambda in0, in1, s0, s1, imm2: (np.abs(in0) * s0 < np.abs(in1)).astype(np.float32))
OP_C2 = _register_custom_op(
    "CANNY_C2", (_ABS1 * C0) <= _ABS0,
    lambda in0, in1, s0, s1, imm2: (np.abs(in1) * s0 <= np.abs(in0)).astype(np.float32))
OP_KEEP = _register_custom_op(
    "CANNY_KEEP", (Src0 >= Src1) & (Src0 > C0),
    lambda in0, in1, s0, s1, imm2: ((in0 >= in1) & (in0 > s0)).astype(np.float32))

H = W = 1024
NCORES = 8
IMGS_PER_CORE = 4
TILE_STARTS = [0, 120, 240, 360, 480, 600, 720, 840, 904]
NKEEP = 120
HALO = 4  # partitions 0..3 hold rows r0-4..r0-1
T2 = float(np.float32(1.0 + np.sqrt(2.0)))  # tan(67.5 deg)
THR = 20.0

# ----------------------------------------------------------------------------
# band construction (host, float64 -> fp16 hi/lo taps identical to reference)
# ----------------------------------------------------------------------------

def _gauss5_f64():
    x = np.arange(5.0) - 2.0
    k = np.exp(-(x ** 2) / 18.0)
    return k / k.sum()

G64 = _gauss5_f64()
COL_SMOOTH = np.array([1.0, 2.0, 1.0])
COL_DIFF = np.array([-1.0, 0.0, 1.0])


def _row_of(p, r0):
    return r0 + p - HALO


def _part_of(v, r0):
    d = v - r0 + HALO
    assert 0 <= d < 128, (v, r0)
    return d


def _reflect(v):
    if v < 0:
        return -v
    if v > H - 1:
        return 2 * (H - 1) - v
    return v


def _blur_bands(r0):
    """10 fp16 [128,128] matrices: hi then lo for dx=-2..2.

    Column j = output partition (blur row r0+j-4); rows = input partitions.
    Valid outputs: partitions 2..125 whose row is inside the image."""
    his, los = [], []
    for dx in range(-2, 3):
        B = np.zeros((128, 128), np.float64)
        for j in range(128):
            if not (2 <= j <= 125):
                continue
            v = _row_of(j, r0)
            if not (0 <= v <= H - 1):
                continue
            for dz in range(-2, 3):
                B[j + dz, j] += G64[dz + 2] * G64[dx + 2]
        B32 = B.astype(np.float32)
        BH = B32.astype(np.float16)
        BL = ((B32.astype(np.float64) - BH.astype(np.float64)) * 4096.0).astype(np.float16)
        his.append(BH)
        los.append(BL)
    return his + los


def _sobel_bands(r0):
    """5 fp16 [128,128] integer matrices: SGY(-1,0,+1), SGX(-1), SGX(+1).

    gy first. Sobel reads blur at row offsets -1,0,1 with reflect-101 at
    image edges (the blur values at reflected rows equal the in-tile
    values by symmetry of the gaussian + reflect padding)."""
    mats = []
    for colfilt, rowtaps in ((COL_DIFF, [1.0, 2.0, 1.0]), (COL_SMOOTH, [-1.0, 1.0])):
        dxs = [-1, 0, 1] if colfilt is COL_DIFF else [-1, 1]
        for idx, dx in enumerate(dxs):
            rt = rowtaps[dx + 1] if colfilt is COL_DIFF else rowtaps[idx]
            B = np.zeros((128, 128), np.float64)
            for j in range(128):
                if not (3 <= j <= 124):
                    continue
                v = _row_of(j, r0)
                if not (0 <= v <= H - 1):
                    continue
                for dz in (-1, 0, 1):
                    w = colfilt[dz + 1]
                    if w == 0.0:
                        continue
                    u = _reflect(v + dz)
                    B[_part_of(u, r0), j] += rt * w
            mats.append(B.astype(np.float16))
    return mats


def _neg_ident():
    return (-np.eye(128)).astype(np.float16)


def _shift_mats():
    SUP = np.zeros((128, 128), np.float32)  # U[j] = mag[j-1]
    SDN = np.zeros((128, 128), np.float32)  # D[j] = mag[j+1]
    for j in range(1, 128):
        SUP[j - 1, j] = 1.0
    for j in range(127):
        SDN[j + 1, j] = 1.0
    return SUP, SDN


def _pack_weights():
    """wt16 [128, (3*15+1)*128] fp16, wt32 [128, 2*128] f32."""
    mats16 = []
    for r0 in (TILE_STARTS[0], TILE_STARTS[1], TILE_STARTS[-1]):
        mats16.extend(_blur_bands(r0))
        mats16.extend(_sobel_bands(r0))
    mats16.append(_neg_ident())
    wt16 = np.stack(mats16, 0)
    wt16 = np.transpose(wt16, (1, 0, 2)).reshape(128, -1).copy()
    SUP, SDN = _shift_mats()
    wt32 = np.stack([SUP, SDN], 0)
    wt32 = np.transpose(wt32, (1, 0, 2)).reshape(128, -1).copy()
    return wt16.astype(np.float16), wt32.astype(np.float32)


def _tile_class(ti):
    if ti == 0:
        return 0
    if ti == len(TILE_STARTS) - 1:
        return 2
    return 1


# ----------------------------------------------------------------------------
# kernel builder
# ----------------------------------------------------------------------------

def build_kernel(n_img=IMGS_PER_CORE, tiles=None):
    if tiles is None:
        tiles = list(range(len(TILE_STARTS)))
    AL = mybir.AluOpType
    f32, f16, i16 = mybir.dt.float32, mybir.dt.float16, mybir.dt.int16

    nc = bacc.Bacc("TRN2", target_bir_lowering=False, debug=False)
    xq_d = nc.dram_tensor("xq", [n_img, H, W], f16, kind="ExternalInput").ap()
    xq2_d = nc.dram_tensor("xq2", [n_img, H, W], f16, kind="ExternalInput").ap()
    wt16_d = nc.dram_tensor("wt16", [128, 46 * 128], f16, kind="ExternalInput").ap()
    wt32_d = nc.dram_tensor("wt32", [128, 2 * 128], f32, kind="ExternalInput").ap()
    out_d = nc.dram_tensor("out", [n_img, H, W], f16, kind="ExternalOutput").ap()

    with tile.TileContext(nc) as tc:
        with (
            tc.tile_pool(name="wts", bufs=1) as wp,
            tc.tile_pool(name="io", bufs=2) as iop,
            tc.tile_pool(name="mid", bufs=2) as mp,
            tc.tile_pool(name="nms", bufs=2) as np_,
            tc.tile_pool(name="pa", bufs=1, space="PSUM") as pa,
            tc.tile_pool(name="pg", bufs=1, space="PSUM") as pg,
            tc.tile_pool(name="pc", bufs=2, space="PSUM") as pc,
        ):
            wt16 = wp.tile([128, 46 * 128], f16)
            wt32 = wp.tile([128, 2 * 128], f32)
            for q in range(4):  # split for parallel DMA engines
                s = q * (46 * 128 // 4)
                e = (q + 1) * (46 * 128 // 4)
                nc.sync.dma_start(out=wt16[:, s:e], in_=wt16_d[:, s:e])
            nc.sync.dma_start(out=wt32[:, :], in_=wt32_d[:, :])

            def m16(c, k):  # fp16 matrix k of tile-class c
                s = (c * 15 + k) * 128
                return wt16[:, s:s + 128]

            NEGI = 45 * 128  # shared negative identity

            def m32(k):
                return wt32[:, k * 128:(k + 1) * 128]

            # PE p-state warmup: dummy matmuls on the first-loaded weight
            # chunk while the image tiles stream in (results discarded)
            for _ in range(8):
                ps_w = pc.tile([128, 512], f32, tag="psh")
                nc.tensor.matmul(ps_w[:, :], wt16[:, 0:128],
                                 wt16[:, 0:512], start=True, stop=True)

            for pair in range(n_img // 2):
                i0 = 2 * pair
                for ti in tiles:
                    r0 = TILE_STARTS[ti]
                    cls = _tile_class(ti)

                    # ---- load xq + xq2 tiles for BOTH images of the pair ----
                    xq_t = iop.tile([128, 2, W + 4], f16, tag="img")
                    xq2 = iop.tile([128, 2, W + 4], f16, tag="img2")
                    for s in range(2):
                        for src_d, dst in ((xq_d, xq_t), (xq2_d, xq2)):
                            if ti == 0:
                                nc.sync.dma_start(out=dst[4:128, s, 2:W + 2],
                                                  in_=src_d[i0 + s, 0:124, :])
                                for k in range(4):
                                    nc.sync.dma_start(out=dst[k:k + 1, s, 2:W + 2],
                                                      in_=src_d[i0 + s, 4 - k:5 - k, :])
                            elif ti == len(TILE_STARTS) - 1:
                                nc.sync.dma_start(out=dst[0:124, s, 2:W + 2],
                                                  in_=src_d[i0 + s, r0 - 4:r0 + 120, :])
                                for k in range(4):
                                    nc.sync.dma_start(out=dst[124 + k:125 + k, s, 2:W + 2],
                                                      in_=src_d[i0 + s, 1022 - k:1023 - k, :])
                            else:
                                nc.sync.dma_start(out=dst[:, s, 2:W + 2],
                                                  in_=src_d[i0 + s, r0 - 4:r0 + 124, :])
                    for dst in (xq_t, xq2):  # reflected pad cols (both slots)
                        nc.gpsimd.tensor_copy(dst[:, :, 1:2], dst[:, :, 3:4])
                        nc.gpsimd.tensor_copy(dst[:, :, 0:1], dst[:, :, 4:5])
                        nc.gpsimd.tensor_copy(dst[:, :, W + 2:W + 3], dst[:, :, W:W + 1])
                        nc.gpsimd.tensor_copy(dst[:, :, W + 3:W + 4], dst[:, :, W - 1:W])

                    # ---- blur + hi/lo split per slot ------------------------
                    bh = mp.tile([128, 2, W + 2], f16, tag="bh")
                    bl = mp.tile([128, 2, W + 2], f16, tag="bl")
                    for s in range(2):
                        ps_blur = pa.tile([128, W], f32, tag="pblur")
                        for c0 in (0, 512):
                            for dxi, dx in enumerate((-2, -1, 0, 1, 2)):
                                rhs = xq_t[:, s, c0 + 2 + dx: c0 + 2 + dx + 512]
                                rhs2 = xq2[:, s, c0 + 2 + dx: c0 + 2 + dx + 512]
                                nc.tensor.matmul(ps_blur[:, c0:c0 + 512], m16(cls, dxi),
                                                 rhs, start=(dxi == 0), stop=False)
                                nc.tensor.matmul(ps_blur[:, c0:c0 + 512], m16(cls, 5 + dxi),
                                                 rhs2, start=False, stop=False)
                        nc.scalar.copy(bh[:, s, 1:W + 1], ps_blur[:, :])
                        for c0 in (0, 512):
                            nc.tensor.matmul(ps_blur[:, c0:c0 + 512],
                                             wt16[:, NEGI:NEGI + 128],
                                             bh[:, s, 1 + c0: 1 + c0 + 512],
                                             start=False, stop=True)
                        nc.scalar.copy(bl[:, s, 1:W + 1], ps_blur[:, :])
                    for t in (bh, bl):  # blur col reflect, both slots
                        nc.gpsimd.tensor_copy(t[:, :, 0:1], t[:, :, 2:3])
                        nc.gpsimd.tensor_copy(t[:, :, W + 1:W + 2], t[:, :, W - 1:W])

                    # ---- sobel: gy pair first, evac, then gx pair -----------
                    ps_gy = pg.tile([128, 2, W], f32, tag="pg")
                    for s in range(2):
                        for c0 in (0, 512):
                            n = 0
                            for k in (0, 1, 2):  # SGY dx=-1,0,+1
                                for src2 in (bh, bl):
                                    dx = k - 1
                                    nc.tensor.matmul(
                                        ps_gy[:, s, c0:c0 + 512], m16(cls, 10 + k),
                                        src2[:, s, c0 + 1 + dx: c0 + 1 + dx + 512],
                                        start=(n == 0), stop=(n == 5))
                                    n += 1
                    gys = mp.tile([128, 2, W], f32, tag="gys")
                    nc.scalar.copy(gys[:, 0, :], ps_gy[:, 0, :])
                    nc.scalar.copy(gys[:, 1, :], ps_gy[:, 1, :])

                    ps_gx = pg.tile([128, 2, W], f32, tag="pg")
                    for s in range(2):
                        for c0 in (0, 512):
                            n = 0
                            for ki, dx in ((3, -1), (4, 1)):  # SGX
                                for src2 in (bh, bl):
                                    nc.tensor.matmul(
                                        ps_gx[:, s, c0:c0 + 512], m16(cls, 10 + ki),
                                        src2[:, s, c0 + 1 + dx: c0 + 1 + dx + 512],
                                        start=(n == 0), stop=(n == 3))
                                    n += 1

                    # ---- mag FIRST (unblocks PE shift matmuls asap) ---------
                    mag = np_.tile([128, 2, W + 2], f32, tag="mag")
                    nc.gpsimd.memset(mag[:, :, 0:1], 0.0)
                    nc.gpsimd.memset(mag[:, :, W + 1:W + 2], 0.0)
                    nc.vector._custom_dve(OP_MAG, out=mag[:, :, 1:W + 1], in0=gys[:, :, :],
                                          in1=ps_gx[:, :, :])

                    # ---- row-shifted mag: fp32 band matmuls, 512 chunks -----
                    usb = np_.tile([128, 2, W + 2], f32, tag="usb")
                    dsb = np_.tile([128, 2, W + 2], f32, tag="dsb")
                    for t in (usb, dsb):
                        nc.gpsimd.memset(t[:, :, 0:1], 0.0)
                        nc.gpsimd.memset(t[:, :, W + 1:W + 2], 0.0)
                    for s in range(2):
                        for c0 in (0, 512):
                            for k, dst in ((0, usb), (1, dsb)):
                                ps_s = pc.tile([128, 512], f32, tag="psh")
                                nc.tensor.matmul(ps_s[:, :], m32(k),
                                                 mag[:, s, 1 + c0:1 + c0 + 512],
                                                 start=True, stop=True)
                                nc.scalar.copy(dst[:, s, 1 + c0:1 + c0 + 512], ps_s[:, :])

                    # ---- masks + mew fill the DVE while PE does shifts ------
                    posm = np_.tile([128, 2, W], f16, tag="posm")
                    c0m = np_.tile([128, 2, W], f16, tag="c0m")
                    c2m = np_.tile([128, 2, W], f16, tag="c2m")
                    mew = np_.tile([128, 2, W], f32, tag="mew")
                    nc.vector.tensor_tensor(mew[:, :, :], mag[:, :, 0:W],
                                            mag[:, :, 2:W + 2], AL.max)
                    nc.vector._custom_dve(OP_POS, out=posm[:, :, :], in0=gys[:, :, :],
                                          in1=ps_gx[:, :, :])
                    nc.vector._custom_dve(OP_C0, out=c0m[:, :, :], in0=gys[:, :, :],
                                          in1=ps_gx[:, :, :], s0=T2)
                    nc.vector._custom_dve(OP_C2, out=c2m[:, :, :], in0=gys[:, :, :],
                                          in1=ps_gx[:, :, :], s0=T2)

                    # ---- NMS pair maxes + bin select ------------------------
                    M = np_.tile([128, 2, W], f32, tag="M")
                    mnesw = np_.tile([128, 2, W], f32, tag="mnesw")
                    mns = np_.tile([128, 2, W], f32, tag="mns")
                    nc.vector.tensor_tensor(M[:, :, :], usb[:, :, 0:W],
                                            dsb[:, :, 2:W + 2], AL.max)
                    nc.vector.tensor_tensor(mnesw[:, :, :], usb[:, :, 2:W + 2],
                                            dsb[:, :, 0:W], AL.max)
                    nc.vector.tensor_tensor(mns[:, :, :], usb[:, :, 1:W + 1],
                                            dsb[:, :, 1:W + 1], AL.max)
                    nc.vector.copy_predicated(M[:, :, :], posm.bitcast(i16)[:, :, :],
                                              mnesw[:, :, :])
                    nc.vector.copy_predicated(M[:, :, :], c2m.bitcast(i16)[:, :, :],
                                              mns[:, :, :])
                    nc.vector.copy_predicated(M[:, :, :], c0m.bitcast(i16)[:, :, :],
                                              mew[:, :, :])

                    # ---- threshold + output (fp16 {0,1}) --------------------
                    keep = iop.tile([128, 2, W], f16, tag="keep")
                    nc.vector._custom_dve(OP_KEEP, out=keep[:, :, :],
                                          in0=mag[:, :, 1:W + 1],
                                          in1=M[:, :, :], s0=THR)
                    for s in range(2):
                        nc.scalar.dma_start(out=out_d[i0 + s, r0:r0 + NKEEP, :],
                                            in_=keep[HALO:HALO + NKEEP, s, :])

    nc.compile()
    return nc


_CACHE = {}


def _get_kernel(n_img):
    if n_img not in _CACHE:
        _CACHE[n_img] = (build_kernel(n_img), *_pack_weights())
    return _CACHE[n_img]


def kernel(image: np.ndarray) -> np.ndarray:
    image = np.asarray(image)
    b = image.shape[0]
    assert image.shape == (b, 1, H, W)
    per = b // NCORES
    assert per * NCORES == b
    nc, wt16, wt32 = _get_kernel(per)
    # host-side quantize: exact integers 0..255, fp16-representable;
    # xq2 = xq * 2^-12 is an exact exponent shift
    xq = np.floor(image[:, 0].astype(np.float32) * np.float32(255.0)).astype(np.float16)
    xq2 = (xq.astype(np.float32) * np.float32(2.0 ** -12)).astype(np.float16)
    in_maps = []
    for c in range(NCORES):
        in_maps.append({
            "xq": np.ascontiguousarray(xq[c * per:(c + 1) * per]),
            "xq2": np.ascontiguousarray(xq2[c * per:(c + 1) * per]),
            "wt16": wt16,
            "wt32": wt32,
        })
    res = bass_utils.run_bass_kernel_spmd(nc, in_maps, core_ids=list(range(NCORES)))
    out = np.empty((b, 1, H, W), np.float32)
    for c in range(NCORES):
        out[c * per:(c + 1) * per, 0] = res.results[c]["out"].astype(np.float32)
    return out


# revision 28
# speedup vs baseline: 1.1676x; 1.1676x over previous
"""Canny edge detector (32,1,1024,1024) on 8 Trainium2 NeuronCores.

Strategy (per core: 4 images, data-parallel over batch; ~566us/core HW):
  - Host-side quantize: xq = floor(255*img) as fp16 (exact integers 0..255),
    fed as the kernel input (halves input DMA). Output is fp16 {0,1}, cast
    to fp32 on host (halves output DMA).
  - Row-tiles of 128 partitions: partition p holds image row r0+p-4
    (4-row top halo in partitions 0..3, so a mid tile is ONE contiguous
    DMA of 128 rows). Output rows r0..r0+119 live in partitions 4..123.
  - Blur = 10 accumulated band matmuls per 512-col half (fp16 hi/lo weight
    split, fp32 PSUM -> near-exact). Blur splits to an fp16 hi/lo pair via
    a PE trick: bh = ACT copy(psum); psum -= I@bh; bl = ACT copy(psum).
  - Sobel gy first (6 matmuls/half) -> gys = ACT evac -> gx reuses the
    same PSUM banks (4 matmuls/half). Custom DVE ops read (gys SBUF,
    ps_gx PSUM): mag, posm, c0m, c2m.
  - NMS in fp32 (fp16 NMS measured 11k+ flips vs ~4.4k budget): mag row
    shifts via fp32 identity-band matmuls in 512-col chunks with their own
    1-bank PSUM pool, ACT evacuation; pair maxes + 3 copy_predicated bin
    selects + fused keep custom op (fp16 {0,1} out).
  - Engine placement: ACT: all PSUM evacuations + out-DMA trigger. Pool:
    reflect pad columns + edge memsets only (HW pool is slow at bulk
    elementwise work). DVE: the 2-tensor fp32 NMS chain only; xq2 comes
    precomputed from the host so the PE never waits on the DVE queue.
  - DVE emission order matters (engines are in-order): mag is emitted
    FIRST so the PE shift matmuls unblock after one DVE op; the mask ops
    and mew then fill the DVE while the PE runs the fp32 shifts.
  - PSUM: blur pool bufs=2 (4 banks) + gy/gx shared pool (2) + shift
    chunks bufs=2 (2) = 8 banks exactly.
"""

import numpy as np

import concourse.bacc as bacc
import concourse.tile as tile
import concourse.mybir as mybir
from concourse import bass_utils
from concourse import dve_ops as _DO
from concourse.dve_spec import Spec, Src0, Src1, C0, Zero, maxx, lower as _dve_lower
from concourse.dve_uop import DveOpSpec as _DveOpSpec


def _register_custom_op(name, body, ref):
    """Runtime-register a fused DVE op (sha self-computed, v3/TRN2)."""
    if name in _DO._SUB_OPCODE_FOR_NAME:
        return next(op for op in _DO.OPS if op.name == name)
    op = _DO.DveOp(name, Spec(body=body, reference=ref), subdim=False, uops_sha={})
    _DO.OPS.append(op)
    _DO.CUSTOM_DVE_SPECS[name] = op.spec
    _DO._SUB_OPCODE_FOR_NAME[name] = _DO._CUSTOM_DVE_ROW_BASE + len(_DO.OPS) - 1
    for ver in ("v3",):
        compiled = _DveOpSpec(
            name=name,
            opcode=_DO.get_dve_sub_opcode(name),
            uops=_dve_lower(op.spec, ver=ver),
            rd1_en=True,
        )
        op.uops_sha[ver] = compiled.sha(ver)
    return op


_ABS0 = maxx(Src0, Zero - Src0)
_ABS1 = maxx(Src1, Zero - Src1)
OP_MAG = _register_custom_op(
    "CANNY_MAG", _ABS0 + _ABS1,
    lambda in0, in1, s0, s1, imm2: np.abs(in0) + np.abs(in1))
OP_POS = _register_custom_op(
    "CANNY_POS", (Src0 * Src1) > Zero,
    lambda in0, in1, s0, s1, imm2: (in0 * in1 > 0).astype(np.float32))
OP_C0 = _register_custom_op(
    "CANNY_C0", (_ABS0 * C0) < _ABS1,
    lambda in0, in1, s0, s1, imm2: (np.abs(in0) * s0 < np.abs(in1)).astype(np.float32))
OP_C2 = _register_custom_op(
    "CANNY_C2", (_ABS1 * C0) <= _ABS0,
    lambda in0, in1, s0, s1, imm2: (np.abs(in1) * s0 <= np.abs(in0)).astype(np.float32))
OP_KEEP = _register_custom_op(
    "CANNY_KEEP", (Src0 >= Src1) & (Src0 > C0),
    lambda in0, in1, s0, s1, imm2: ((in0 >= in1) & (in0 > s0)).astype(np.float32))

H = W = 1024
NCORES = 8
IMGS_PER_CORE = 4
TILE_STARTS = [0, 120, 240, 360, 480, 600, 720, 840, 904]
NKEEP = 120
HALO = 4  # partitions 0..3 hold rows r0-4..r0-1
T2 = float(np.float32(1.0 + np.sqrt(2.0)))  # tan(67.5 deg)
THR = 20.0

# ----------------------------------------------------------------------------
# band construction (host, float64 -> fp16 hi/lo taps identical to reference)
# ----------------------------------------------------------------------------

def _gauss5_f64():
    x = np.arange(5.0) - 2.0
    k = np.exp(-(x ** 2) / 18.0)
    return k / k.sum()

G64 = _gauss5_f64()
COL_SMOOTH = np.array([1.0, 2.0, 1.0])
COL_DIFF = np.array([-1.0, 0.0, 1.0])


def _row_of(p, r0):
    return r0 + p - HALO


def _part_of(v, r0):
    d = v - r0 + HALO
    assert 0 <= d < 128, (v, r0)
    return d


def _reflect(v):
    if v < 0:
        return -v
    if v > H - 1:
        return 2 * (H - 1) - v
    return v


def _blur_bands(r0):
    """10 fp16 [128,128] matrices: hi then lo for dx=-2..2.

    Column j = output partition (blur row r0+j-4); rows = input partitions.
    Valid outputs: partitions 2..125 whose row is inside the image."""
    his, los = [], []
    for dx in range(-2, 3):
        B = np.zeros((128, 128), np.float64)
        for j in range(128):
            if not (2 <= j <= 125):
                continue
            v = _row_of(j, r0)
            if not (0 <= v <= H - 1):
                continue
            for dz in range(-2, 3):
                B[j + dz, j] += G64[dz + 2] * G64[dx + 2]
        B32 = B.astype(np.float32)
        BH = B32.astype(np.float16)
        BL = ((B32.astype(np.float64) - BH.astype(np.float64)) * 4096.0).astype(np.float16)
        his.append(BH)
        los.append(BL)
    return his + los


def _sobel_bands(r0):
    """5 fp16 [128,128] integer matrices: SGY(-1,0,+1), SGX(-1), SGX(+1).

    gy first. Sobel reads blur at row offsets -1,0,1 with reflect-101 at
    image edges (the blur values at reflected rows equal the in-tile
    values by symmetry of the gaussian + reflect padding)."""
    mats = []
    for colfilt, rowtaps in ((COL_DIFF, [1.0, 2.0, 1.0]), (COL_SMOOTH, [-1.0, 1.0])):
        dxs = [-1, 0, 1] if colfilt is COL_DIFF else [-1, 1]
        for idx, dx in enumerate(dxs):
            rt = rowtaps[dx + 1] if colfilt is COL_DIFF else rowtaps[idx]
            B = np.zeros((128, 128), np.float64)
            for j in range(128):
                if not (3 <= j <= 124):
                    continue
                v = _row_of(j, r0)
                if not (0 <= v <= H - 1):
                    continue
                for dz in (-1, 0, 1):
                    w = colfilt[dz + 1]
                    if w == 0.0:
                        continue
                    u = _reflect(v + dz)
                    B[_part_of(u, r0), j] += rt * w
            mats.append(B.astype(np.float16))
    return mats


def _neg_ident():
    return (-np.eye(128)).astype(np.float16)


def _shift_mats():
    SUP = np.zeros((128, 128), np.float32)  # U[j] = mag[j-1]
    SDN = np.zeros((128, 128), np.float32)  # D[j] = mag[j+1]
    for j in range(1, 128):
        SUP[j - 1, j] = 1.0
    for j in range(127):
        SDN[j + 1, j] = 1.0
    return SUP, SDN


def _pack_weights():
    """wt16 [128, (3*15+1)*128] fp16, wt32 [128, 2*128] f32."""
    mats16 = []
    for r0 in (TILE_STARTS[0], TILE_STARTS[1], TILE_STARTS[-1]):
        mats16.extend(_blur_bands(r0))
        mats16.extend(_sobel_bands(r0))
    mats16.append(_neg_ident())
    wt16 = np.stack(mats16, 0)
    wt16 = np.transpose(wt16, (1, 0, 2)).reshape(128, -1).copy()
    SUP, SDN = _shift_mats()
    wt32 = np.stack([SUP, SDN], 0)
    wt32 = np.transpose(wt32, (1, 0, 2)).reshape(128, -1).copy()
    return wt16.astype(np.float16), wt32.astype(np.float32)


def _tile_class(ti):
    if ti == 0:
        return 0
    if ti == len(TILE_STARTS) - 1:
        return 2
    return 1


# ----------------------------------------------------------------------------
# kernel builder
# ----------------------------------------------------------------------------

def build_kernel(n_img=IMGS_PER_CORE, tiles=None):
    if tiles is None:
        tiles = list(range(len(TILE_STARTS)))
    AL = mybir.AluOpType
    f32, f16, i16 = mybir.dt.float32, mybir.dt.float16, mybir.dt.int16

    nc = bacc.Bacc("TRN2", target_bir_lowering=False, debug=False)
    xq_d = nc.dram_tensor("xq", [n_img, H, W], f16, kind="ExternalInput").ap()
    xq2_d = nc.dram_tensor("xq2", [n_img, H, W], f16, kind="ExternalInput").ap()
    wt16_d = nc.dram_tensor("wt16", [128, 46 * 128], f16, kind="ExternalInput").ap()
    wt32_d = nc.dram_tensor("wt32", [128, 2 * 128], f32, kind="ExternalInput").ap()
    out_d = nc.dram_tensor("out", [n_img, H, W], f16, kind="ExternalOutput").ap()

    with tile.TileContext(nc) as tc:
        with (
            tc.tile_pool(name="wts", bufs=1) as wp,
            tc.tile_pool(name="io", bufs=5) as iop,
            tc.tile_pool(name="mid", bufs=3) as mp,
            tc.tile_pool(name="nms", bufs=3) as np_,
            tc.tile_pool(name="pa", bufs=2, space="PSUM") as pa,
            tc.tile_pool(name="pg", bufs=1, space="PSUM") as pg,
            tc.tile_pool(name="pc", bufs=2, space="PSUM") as pc,
        ):
            wt16 = wp.tile([128, 46 * 128], f16)
            wt32 = wp.tile([128, 2 * 128], f32)
            for q in range(4):  # split for parallel DMA engines
                s = q * (46 * 128 // 4)
                e = (q + 1) * (46 * 128 // 4)
                nc.sync.dma_start(out=wt16[:, s:e], in_=wt16_d[:, s:e])
            nc.sync.dma_start(out=wt32[:, :], in_=wt32_d[:, :])

            def m16(c, k):  # fp16 matrix k of tile-class c
                s = (c * 15 + k) * 128
                return wt16[:, s:s + 128]

            NEGI = 45 * 128  # shared negative identity

            def m32(k):
                return wt32[:, k * 128:(k + 1) * 128]

            # PE p-state warmup: dummy matmuls on the first-loaded weight
            # chunk while the image tiles stream in (results discarded)
            for _ in range(8):
                ps_w = pc.tile([128, 512], f32, tag="psh")
                nc.tensor.matmul(ps_w[:, :], wt16[:, 0:128],
                                 wt16[:, 0:512], start=True, stop=True)

            for i in range(n_img):
                for ti in tiles:
                    r0 = TILE_STARTS[ti]
                    cls = _tile_class(ti)

                    # ---- load xq + xq2 tiles (fp16): partition p = img row
                    # r0+p-4, cols 2..1025 = img cols 0..1023, reflect pads.
                    xq_t = iop.tile([128, W + 4], f16, tag="img")
                    xq2 = iop.tile([128, W + 4], f16, tag="img2")
                    for src_d, dst in ((xq_d, xq_t), (xq2_d, xq2)):
                        if ti == 0:
                            nc.sync.dma_start(out=dst[4:128, 2:W + 2],
                                              in_=src_d[i, 0:124, :])
                            for k in range(4):  # rows -4..-1 = rows 4,3,2,1
                                nc.sync.dma_start(out=dst[k:k + 1, 2:W + 2],
                                                  in_=src_d[i, 4 - k:5 - k, :])
                        elif ti == len(TILE_STARTS) - 1:
                            nc.sync.dma_start(out=dst[0:124, 2:W + 2],
                                              in_=src_d[i, r0 - 4:r0 + 120, :])
                            for k in range(4):  # rows 1024..1027 = 1022..1019
                                nc.sync.dma_start(out=dst[124 + k:125 + k, 2:W + 2],
                                                  in_=src_d[i, 1022 - k:1023 - k, :])
                        else:
                            nc.sync.dma_start(out=dst[:, 2:W + 2],
                                              in_=src_d[i, r0 - 4:r0 + 124, :])
                        # reflected pad columns (pool)
                        nc.gpsimd.tensor_copy(dst[:, 1:2], dst[:, 3:4])
                        nc.gpsimd.tensor_copy(dst[:, 0:1], dst[:, 4:5])
                        nc.gpsimd.tensor_copy(dst[:, W + 2:W + 3], dst[:, W:W + 1])
                        nc.gpsimd.tensor_copy(dst[:, W + 3:W + 4], dst[:, W - 1:W])

                    # ---- blur: 5 dx x {hi,lo} accumulated band matmuls ------
                    ps_blur = pa.tile([128, W], f32, tag="pblur")
                    for c0 in (0, 512):
                        for dxi, dx in enumerate((-2, -1, 0, 1, 2)):
                            rhs = xq_t[:, c0 + 2 + dx: c0 + 2 + dx + 512]
                            rhs2 = xq2[:, c0 + 2 + dx: c0 + 2 + dx + 512]
                            nc.tensor.matmul(ps_blur[:, c0:c0 + 512], m16(cls, dxi),
                                             rhs, start=(dxi == 0), stop=False)
                            nc.tensor.matmul(ps_blur[:, c0:c0 + 512], m16(cls, 5 + dxi),
                                             rhs2, start=False, stop=False)

                    # ---- split blur -> fp16 hi/lo via PE accumulate ---------
                    bh = mp.tile([128, W + 2], f16, tag="bh")  # col m = blur col m-1
                    bl = mp.tile([128, W + 2], f16, tag="bl")
                    nc.scalar.copy(bh[:, 1:W + 1], ps_blur[:, :])
                    for c0 in (0, 512):
                        nc.tensor.matmul(ps_blur[:, c0:c0 + 512],
                                         wt16[:, NEGI:NEGI + 128],
                                         bh[:, 1 + c0: 1 + c0 + 512],
                                         start=False, stop=True)
                    nc.scalar.copy(bl[:, 1:W + 1], ps_blur[:, :])
                    for t in (bh, bl):  # blur col reflect: -1 = 1, 1024 = 1022
                        nc.gpsimd.tensor_copy(t[:, 0:1], t[:, 2:3])
                        nc.gpsimd.tensor_copy(t[:, W + 1:W + 2], t[:, W - 1:W])

                    # ---- sobel: gy first, evac, then gx in the same banks ---
                    ps_gy = pg.tile([128, W], f32, tag="pg")
                    for c0 in (0, 512):
                        n = 0
                        for k in (0, 1, 2):  # SGY dx=-1,0,+1
                            for src in (bh, bl):
                                dx = k - 1
                                nc.tensor.matmul(
                                    ps_gy[:, c0:c0 + 512], m16(cls, 10 + k),
                                    src[:, c0 + 1 + dx: c0 + 1 + dx + 512],
                                    start=(n == 0), stop=(n == 5))
                                n += 1
                    gys = mp.tile([128, W], f32, tag="gys")
                    nc.scalar.copy(gys[:, :], ps_gy[:, :])

                    ps_gx = pg.tile([128, W], f32, tag="pg")
                    for c0 in (0, 512):
                        n = 0
                        for ki, dx in ((3, -1), (4, 1)):  # SGX
                            for src in (bh, bl):
                                nc.tensor.matmul(
                                    ps_gx[:, c0:c0 + 512], m16(cls, 10 + ki),
                                    src[:, c0 + 1 + dx: c0 + 1 + dx + 512],
                                    start=(n == 0), stop=(n == 3))
                                n += 1
                    # ---- mag FIRST (unblocks PE shift matmuls asap) ---------
                    mag = np_.tile([128, W + 2], f32, tag="mag")  # col m = img col m-1
                    nc.gpsimd.memset(mag[:, 0:1], 0.0)
                    nc.gpsimd.memset(mag[:, W + 1:W + 2], 0.0)
                    nc.vector._custom_dve(OP_MAG, out=mag[:, 1:W + 1], in0=gys[:, :],
                                          in1=ps_gx[:, :])

                    # ---- row-shifted mag: fp32 band matmuls, 512 chunks -----
                    usb = np_.tile([128, W + 2], f32, tag="usb")
                    dsb = np_.tile([128, W + 2], f32, tag="dsb")
                    for t in (usb, dsb):
                        nc.gpsimd.memset(t[:, 0:1], 0.0)
                        nc.gpsimd.memset(t[:, W + 1:W + 2], 0.0)
                    for c0 in (0, 512):
                        for k, dst in ((0, usb), (1, dsb)):
                            ps_s = pc.tile([128, 512], f32, tag="psh")
                            nc.tensor.matmul(ps_s[:, :], m32(k),
                                             mag[:, 1 + c0:1 + c0 + 512],
                                             start=True, stop=True)
                            nc.scalar.copy(dst[:, 1 + c0:1 + c0 + 512], ps_s[:, :])

                    # ---- masks + mew fill the DVE while PE does shifts ------
                    posm = np_.tile([128, W], f16, tag="posm")
                    c0m = np_.tile([128, W], f16, tag="c0m")
                    c2m = np_.tile([128, W], f16, tag="c2m")
                    mew = np_.tile([128, W], f32, tag="mew")
                    nc.vector.tensor_tensor(mew[:, :], mag[:, 0:W], mag[:, 2:W + 2], AL.max)
                    nc.vector._custom_dve(OP_POS, out=posm[:, :], in0=gys[:, :],
                                          in1=ps_gx[:, :])
                    nc.vector._custom_dve(OP_C0, out=c0m[:, :], in0=gys[:, :],
                                          in1=ps_gx[:, :], s0=T2)
                    nc.vector._custom_dve(OP_C2, out=c2m[:, :], in0=gys[:, :],
                                          in1=ps_gx[:, :], s0=T2)

                    # ---- NMS pair maxes + bin select ------------------------
                    M = np_.tile([128, W], f32, tag="M")
                    mnesw = np_.tile([128, W], f32, tag="mnesw")
                    mns = np_.tile([128, W], f32, tag="mns")
                    nc.vector.tensor_tensor(M[:, :], usb[:, 0:W], dsb[:, 2:W + 2], AL.max)
                    nc.vector.tensor_tensor(mnesw[:, :], usb[:, 2:W + 2], dsb[:, 0:W], AL.max)
                    nc.vector.tensor_tensor(mns[:, :], usb[:, 1:W + 1], dsb[:, 1:W + 1], AL.max)
                    nc.vector.copy_predicated(M[:, :], posm.bitcast(i16)[:, :], mnesw[:, :])
                    nc.vector.copy_predicated(M[:, :], c2m.bitcast(i16)[:, :], mns[:, :])
                    nc.vector.copy_predicated(M[:, :], c0m.bitcast(i16)[:, :], mew[:, :])

                    # ---- threshold + output (fp16 {0,1}) --------------------
                    keep = iop.tile([128, W], f16, tag="keep")
                    nc.vector._custom_dve(OP_KEEP, out=keep[:, :], in0=mag[:, 1:W + 1],
                                          in1=M[:, :], s0=THR)
                    # out-DMA from ACT so SP never head-of-line blocks loads
                    nc.scalar.dma_start(out=out_d[i, r0:r0 + NKEEP, :],
                                        in_=keep[HALO:HALO + NKEEP, :])

    nc.compile()
    return nc


_CACHE = {}


def _get_kernel(n_img):
    if n_img not in _CACHE:
        _CACHE[n_img] = (build_kernel(n_img), *_pack_weights())
    return _CACHE[n_img]


def kernel(image: np.ndarray) -> np.ndarray:
    image = np.asarray(image)
    b = image.shape[0]
    assert image.shape == (b, 1, H, W)
    per = b // NCORES
    assert per * NCORES == b
    nc, wt16, wt32 = _get_kernel(per)
    # host-side quantize: exact integers 0..255, fp16-representable;
    # xq2 = xq * 2^-12 is an exact exponent shift
    xq = np.floor(image[:, 0].astype(np.float32) * np.float32(255.0)).astype(np.float16)
    xq2 = (xq.astype(np.float32) * np.float32(2.0 ** -12)).astype(np.float16)
    in_maps = []
    for c in range(NCORES):
        in_maps.append({
            "xq": np.ascontiguousarray(xq[c * per:(c + 1) * per]),
            "xq2": np.ascontiguousarray(xq2[c * per:(c + 1) * per]),
            "wt16": wt16,
            "wt32": wt32,
        })
    res = bass_utils.run_bass_kernel_spmd(nc, in_maps, core_ids=list(range(NCORES)))
    out = np.empty((b, 1, H, W), np.float32)
    for c in range(NCORES):
        out[c * per:(c + 1) * per, 0] = res.results[c]["out"].astype(np.float32)
    return out
